# revision 7
# baseline (speedup 1.0000x reference)
"""DDSP core synthesizer kernel for Trainium2 (8 NeuronCores, data-parallel).

Reference computation (per row of B=32, T=64000):
    f0_hz = 20*exp(f0); phase = cumsum(2*pi*f0_hz/SR)
    hw    = sum_k sin(phase*k)/k   (k = 1..60)
    audio = mix*hw*loud + (1-mix)*noise*loud;  out = audio / (max|audio| + 1e-6)

Device algorithm (everything in "turns" = phase/2pi), natural layout
[128 partitions = 4 rows x 32 blocks, 2000 free = time-in-block]:
    inc  = exp(f0 + ln(20/SR))                    [ACT Exp]
    u    = blocked cumsum + triangular-matmul offs [DVE scan + PE]
    u1   = frac(u)  in [-0.5, 0.5]                 [custom DVE FRAC_AFFINE]
    per harmonic k = 1..60:
        v_k = frac(k*u1)                           [custom DVE FRAC_AFFINE, 1 op]
        s_k = sin(2pi*v_k) -> bf16                 [ACT Sin, scale=2pi]
        hw += diag(1/k) @ s_k                      [PE accumulate in PSUM]
    epilogue: audio = A*hw + B with A = loud*mix, B = loud*noise*(1-mix)
              (A, B precomputed on GpSimd during the k-loop);
              peak-normalize per row (free-reduce + 32x32 transpose trick).

The custom DVE op (v = x - ((x+M)-M), x = Src0*C0 + C1, M = magic rint
constant) replaces the baseline's two extra PE passes (f16 x-gen matmul +
negi-subtract matmul) and the PSUM-sourced DVE rint, cutting PE work per
pass from 3 matmul sweeps to 1 and DVE work to a single SBUF-sourced op.

Sharding: pure data parallel, 4 rows per core, SPMD on cores 0-7.
"""

import sys

sys.path.insert(0, "/opt/trn_rl_repo")

import numpy as np
import ml_dtypes
from contextlib import ExitStack

import concourse.bass as bass
import concourse.tile as tile
from concourse import bacc, mybir
from concourse import bass_utils

f32 = np.float32
dt = mybir.dt

SR = 44100.0
H = 60                      # harmonics
B, T = 32, 64000
NCORES = 8
RPC = B // NCORES           # rows per core = 4
P = 128                     # SBUF partitions
FD = T * RPC // P           # free dim of master tiles = 2000
BPR = P // RPC              # blocks per row = 32
PI = float(np.pi)
MAGIC = float(1.5 * 2.0 ** 23)
EXP_BIAS = float(np.log(20.0 / SR))

_cache = {}


def _register_frac_op():
    """Register FRAC_AFFINE_ANT: out = x - ((x + M) - M), x = Src0*C0 + C1.

    C0/C1 are per-partition scalars (or literals), M = imm2 (the fp32 magic
    rint constant). 5 ALU stages, well under the 8-stage DVE budget."""
    if "op" in _cache.get("frac", {}):
        return _cache["frac"]["op"]
    from concourse import dve_ops
    from concourse.dve_spec import Spec, Src0, C0, C1, C2, lower
    from concourse.dve_spec import _has_src1 as has_src1
    from concourse.dve_uop import DveOpSpec
    from concourse.dve_table_gen import dve_ver_for

    name = "FRAC_AFFINE_ANT"

    def ref(in0, in1, s0, s1, imm2):
        x = (in0.astype(f32) * f32(s0) + f32(s1)).astype(f32)
        t = (x + f32(imm2)).astype(f32)
        w = (t - f32(imm2)).astype(f32)
        return (x - w).astype(f32)

    x = Src0 * C0 + C1
    spec = Spec(body=x - ((x + C2) - C2), reference=ref)

    if name not in dve_ops._SUB_OPCODE_FOR_NAME:
        row = max(dve_ops._SUB_OPCODE_FOR_NAME.values()) + 1
        assert row < 0x20
        dve_ops._SUB_OPCODE_FOR_NAME[name] = row

    ver = dve_ver_for("TRN2")
    tmp = DveOpSpec(
        name=name,
        opcode=dve_ops.get_dve_sub_opcode(name),
        uops=lower(spec, ver=ver),
        rd1_en=has_src1(spec),
    )
    op = dve_ops.DveOp(name, spec, subdim=False, uops_sha={ver: tmp.sha(ver)})
    if not any(o.name == name for o in dve_ops.OPS):
        dve_ops.OPS.append(op)
    dve_ops.CUSTOM_DVE_SPECS[name] = spec
    _cache["frac"] = {"op": op}
    return op


def _consts():
    # lt: exclusive-prefix matmul weights. offs[m] = sum_k lt[k, m] * totals[k]
    kk, mm_ = np.meshgrid(np.arange(P), np.arange(P), indexing="ij")
    lt = ((kk // BPR == mm_ // BPR) & (kk % BPR < mm_ % BPR)).astype(f32)

    # diags[k] = diag(1/(k+1)) as 128x128 lhsT for the PSUM accumulate
    diags = np.zeros((H, P, P), dtype=np.float64)
    for k in range(H):
        np.fill_diagonal(diags[k], 1.0 / (k + 1))
    diags = diags.astype(ml_dtypes.bfloat16)
    return {"lt": lt, "diags": diags}


def _build():
    frac_op = _register_frac_op()
    nc = bacc.Bacc("TRN2", target_bir_lowering=False, debug=False,
                   enable_asserts=True, num_devices=NCORES)

    f0_d = nc.dram_tensor("f0", [P, FD], dt.float32, kind="ExternalInput")
    loud_d = nc.dram_tensor("loud", [P, FD], dt.float32, kind="ExternalInput")
    mix_d = nc.dram_tensor("mix", [P, FD], dt.float32, kind="ExternalInput")
    noise_d = nc.dram_tensor("noise", [P, FD], dt.float32, kind="ExternalInput")
    lt_d = nc.dram_tensor("lt", [P, P], dt.float32, kind="ExternalInput")
    diags_d = nc.dram_tensor("diags", [H, P, P], dt.bfloat16, kind="ExternalInput")
    out_d = nc.dram_tensor("audio", [P, FD], dt.float32, kind="ExternalOutput")

    AF = mybir.ActivationFunctionType
    ALU = mybir.AluOpType

    with tile.TileContext(nc) as tc, ExitStack() as ctx:
        pool = ctx.enter_context(tc.tile_pool(name="sb", bufs=1))
        vpool = ctx.enter_context(tc.tile_pool(name="vp", bufs=3))
        spool = ctx.enter_context(tc.tile_pool(name="sp", bufs=3))
        hpool = ctx.enter_context(tc.tile_pool(name="hps", bufs=1, space="PSUM"))
        opool = ctx.enter_context(tc.tile_pool(name="ops", bufs=1, space="PSUM"))

        exp_bias = pool.tile([P, 1], dt.float32, tag="cbias_exp")
        nc.vector.memset(exp_bias[:], EXP_BIAS)
        zero_bias = pool.tile([P, 1], dt.float32, tag="cbias_zero")
        nc.vector.memset(zero_bias[:], 0.0)

        # ---- input DMA ----
        f0 = pool.tile([P, FD], dt.float32, tag="scr", bufs=4, name="f0")
        nc.sync.dma_start(f0[:], f0_d.ap())
        lt = pool.tile([P, P], dt.float32)
        nc.gpsimd.dma_start(lt[:], lt_d.ap())
        diags = pool.tile([P, H, P], dt.bfloat16)
        nc.gpsimd.dma_start(diags[:], diags_d.ap().rearrange("k p m -> p k m"))
        loud = pool.tile([P, FD], dt.float32, tag="loud")
        nc.scalar.dma_start(loud[:], loud_d.ap())
        mix = pool.tile([P, FD], dt.float32, tag="mix")
        nc.scalar.dma_start(mix[:], mix_d.ap())
        noise = pool.tile([P, FD], dt.float32, tag="noise")
        nc.scalar.dma_start(noise[:], noise_d.ap())

        # ---- stage 1: phase accumulation (turns) ----
        inc = pool.tile([P, FD], dt.float32, tag="scr", bufs=4, name="inc")
        nc.scalar.activation(inc[:], f0[:], AF.Exp, bias=exp_bias[:, 0:1], scale=1.0)

        local = pool.tile([P, FD], dt.float32, tag="scr", bufs=4, name="local")
        nc.vector.tensor_tensor_scan(local[:], inc[:], inc[:], 0.0,
                                     ALU.add, ALU.bypass)

        offs_ps = opool.tile([P, 1], dt.float32, tag="offs")
        nc.tensor.matmul(offs_ps[:], lt[:], local[:, FD - 1:FD],
                         start=True, stop=True)
        offs = pool.tile([P, 1], dt.float32)
        nc.vector.tensor_copy(offs[:], offs_ps[:])

        # u1 = frac(local + offs) in one custom-DVE op
        u1 = pool.tile([P, FD], dt.float32, tag="u1")
        nc.vector._custom_dve(frac_op, out=u1[:], in0=local[:],
                              s0=1.0, s1=offs[:, 0:1], imm2=MAGIC)

        # ---- epilogue prework on GpSimd (overlaps the k-loop) ----
        # audio = A*hw + B;  A = loud*mix, B = loud*noise*(1-mix)
        A = pool.tile([P, FD], dt.float32, tag="A")
        nc.gpsimd.tensor_tensor(A[:], loud[:], mix[:], ALU.mult)
        ln_ = pool.tile([P, FD], dt.float32, tag="ln")
        nc.gpsimd.tensor_tensor(ln_[:], loud[:], noise[:], ALU.mult)
        lnm = pool.tile([P, FD], dt.float32, tag="lnm")
        nc.gpsimd.tensor_tensor(lnm[:], ln_[:], mix[:], ALU.mult)
        Bt = pool.tile([P, FD], dt.float32, tag="Bt")
        nc.gpsimd.tensor_tensor(Bt[:], ln_[:], lnm[:], ALU.subtract)

        # ---- k-loop: v_k = frac(k*u1); s_k = sin(2pi v_k); hw += s_k/k ----
        hw = hpool.tile([P, 4, 512], dt.float32, tag="hw")
        for k in range(1, H + 1):
            v = vpool.tile([P, FD], dt.float32, tag="v")
            nc.vector._custom_dve(frac_op, out=v[:], in0=u1[:],
                                  s0=float(k), s1=0.0, imm2=MAGIC)
            s = spool.tile([P, FD], dt.bfloat16, tag="s")
            nc.scalar.activation(s[:], v[:], AF.Sin,
                                 bias=zero_bias[:, 0:1], scale=2.0 * PI)
            for qo in range(0, FD, 512):
                qn = min(512, FD - qo)
                nc.tensor.matmul(hw[:, qo // 512, 0:qn],
                                 diags[:, k - 1, :], s[:, qo:qo + qn],
                                 start=(k == 1), stop=(k == H))

        # ---- epilogue: audio = A*hw + B, then per-row peak normalize ----
        hw_flat = hw[:].rearrange("p q f -> p (q f)")[:, 0:FD]
        e1 = pool.tile([P, FD], dt.float32, tag="e1")
        nc.vector.tensor_tensor(e1[:], A[:], hw_flat, ALU.mult)
        audio = pool.tile([P, FD], dt.float32, tag="audio")
        nc.vector.tensor_tensor(audio[:], e1[:], Bt[:], ALU.add)

        # per-row peak: free-dim abs-max then 32x32 block transpose trick
        pk = pool.tile([P, 1], dt.float32, tag="pk")
        nc.vector.tensor_reduce(pk[:], audio[:], axis=mybir.AxisListType.X,
                                op=ALU.max, apply_absolute_value=True)
        pkr = pool.tile([P, 32], dt.float32, tag="pkr")
        nc.vector.tensor_copy(pkr[:], pk[:, 0:1].to_broadcast((P, 32)))
        pkt = pool.tile([P, 32], dt.float32, tag="pkt")
        nc.vector.transpose(pkt[:], pkr[:])
        rowmax = pool.tile([P, 1], dt.float32, tag="rowmax")
        nc.vector.tensor_reduce(rowmax[:], pkt[:],
                                axis=mybir.AxisListType.X, op=ALU.max)
        pke = pool.tile([P, 1], dt.float32, tag="pke")
        nc.vector.tensor_scalar(pke[:], rowmax[:], 1e-6, None, ALU.add)
        rcp = pool.tile([P, 1], dt.float32, tag="rcp")
        nc.vector.reciprocal(rcp[:], pke[:])
        outt = pool.tile([P, FD], dt.float32, tag="outt")
        nc.vector.tensor_scalar(outt[:], audio[:], rcp[:, 0:1], None, ALU.mult)
        nc.sync.dma_start(out_d.ap(), outt[:])

    nc.compile()
    return nc


def kernel(f0, loudness, harmonic_mix, noise):
    if "nc" not in _cache:
        _cache["nc"] = _build()
        _cache["consts"] = _consts()
    nc = _cache["nc"]
    consts = _cache["consts"]

    def shard(a, c):
        return np.ascontiguousarray(
            a[c * RPC:(c + 1) * RPC].astype(f32, copy=False).reshape(P, FD))

    in_maps = []
    for c in range(NCORES):
        in_maps.append({
            "f0": shard(f0, c),
            "loud": shard(loudness, c),
            "mix": shard(harmonic_mix, c),
            "noise": shard(noise, c),
            **consts,
        })

    res = bass_utils.run_bass_kernel_spmd(nc, in_maps, core_ids=list(range(NCORES)))
    outs = [res.results[c]["audio"].reshape(RPC, T) for c in range(NCORES)]
    return np.concatenate(outs, axis=0)
